# revision 25
# baseline (speedup 1.0000x reference)
"""Trainium2 Bass kernel for nn_Attend (l2-distance attention with zero-kv).

Reference computation (per b,h):
    k' = [0; k], v' = [0; v]                       (prepend zero kv)
    scores[i,j] = (2 q_i.k'_j - |q_i|^2 - |k'_j|^2) * (D+2)^-0.5
    causal: j <= i+1 in padded index space
    out = softmax(scores) @ v'

Kernel algebra: softmax is invariant to the per-row constant -scale*|q_i|^2,
so with p~[i,j] = exp(2*scale*q_i.k_j) * exp(-scale*|k_j|^2) and the zero
column contributing exp(0)=1 to the denominator only:
    out_i = (sum_j p~ v_j) / (1 + sum_j p~)

v4 design (vs v3 baseline at 184us):
  * All input staging moved to the HOST: q^T/k^T prepacked bf16 with the
    two heads of a pair stacked block-diagonally on the partition dim
    (K=128 keeps the PE moving operand at full rate), vo = [v*ek | ek]
    with ek = exp(-scale*|k|^2) folded in, and the causal triangle mask.
    Removes all on-device transposes/casts/copies (~40us DVE/gpsimd/DMA).
  * Scores stay transposed ([kv, q]); for diagonal kv blocks the matmul,
    exp and PV are restricted to columns >= 128*r (the fully-masked strip
    is never computed), with per-r dedicated pre-zeroed pt tiles.
  * exp is split across TWO engines: the Scalar/ACT engine (table exp)
    and the DVE via a one-instruction bit-trick:
        bf16_bits(exp(x)) ~= round(128*(log2e*x + 127 - c))
    written as tensor_scalar(out=uint16 view, in0=scores, mult, add).
  * The triangle mask multiply runs on the DVE right after the exp.
  * No on-device softmax division/transpose: the [65, q] accumulator
    (64 v-dims + denominator row) is DMA'd out raw; the host does
    num/(1+den) and the final [d,q]->[q,d] transpose.

Sharding: 32 (b,h) pairs -> 4 heads per core, 8 cores, pure data parallel.
"""

import sys

for _p in ("/opt/trn_rl_repo", "/root/.axon_site"):
    if _p not in sys.path:
        sys.path.insert(0, _p)

import numpy as np

B, H, N, D = 2, 16, 2048, 64
NCORES = 8
HPC = (B * H) // NCORES          # heads per core = 4
NPAIRS = HPC // 2
SCALE = float((D + 2) ** -0.5)   # augmented head dim, matches reference
NB = N // 128                    # kv blocks of 128 = 16
NQT = N // 512                   # q tiles of 512 = 4

# DVE bit-trick exp constants: bits = TS_A * x + TS_B, viewed as bf16
_C_CORR = 0.04303
TS_A = float(2.0 * SCALE * 128.0 / np.log(2.0))
TS_B = float((127.0 - _C_CORR) * 128.0)

_BUILT = {}


def _build(dve_mode="AD", mask_engine="vector", hpc=HPC, n=N):
    """Build + finalize the SPMD Bass program (one core's view).

    dve_mode: per-j engine pattern for the exp ("A" = ACT table exp,
    "D" = DVE bit-trick), cycled over the global j counter.
    """
    import concourse.mybir as mybir
    import concourse.tile as tile
    from concourse import bacc

    f32 = mybir.dt.float32
    bf16 = mybir.dt.bfloat16
    u16 = mybir.dt.uint16
    Exp = mybir.ActivationFunctionType.Exp
    mult = mybir.AluOpType.mult
    add = mybir.AluOpType.add

    npairs = hpc // 2
    nb = n // 128
    nqt = n // 512
    nh = n // 2  # DMA half size

    nc = bacc.Bacc("TRN2", target_bir_lowering=False, debug=False)
    q2_p = nc.declare_dram_parameter("qT2", [npairs, 128, 2, n], bf16, isOutput=False)
    kT_p = nc.declare_dram_parameter("kT", [npairs, 128, n], bf16, isOutput=False)
    vo_p = nc.declare_dram_parameter("vo", [hpc, 128, nb, 65], bf16, isOutput=False)
    tri_p = nc.declare_dram_parameter("tri", [128, 2, 128], bf16, isOutput=False)
    o_p = nc.declare_dram_parameter("out", [npairs, nqt, 65, 1024], f32, isOutput=True)

    with tile.TileContext(nc) as tc:
        with (
            tc.tile_pool(name="const", bufs=1) as constp,
            tc.tile_pool(name="ptd", bufs=1) as ptdp,
            tc.tile_pool(name="pto", bufs=6) as ptop,
            tc.tile_pool(name="fin", bufs=2) as finp,
            tc.tile_pool(name="ps_sp", bufs=3, space="PSUM") as ps_sp,
            tc.tile_pool(name="ps_acc", bufs=1, space="PSUM") as ps_acc,
        ):
            tri = constp.tile([128, 2, 128], bf16, tag="tri")
            kTs, q2s, vos = [], [], []
            # staged DMAs so the first matmuls start early; pair 0 first
            for p in range(npairs):
                kT = constp.tile([128, n], bf16, tag=f"kT{p}", name=f"kT_{p}")
                q2 = constp.tile([128, 2, n], bf16, tag=f"q2{p}", name=f"q2_{p}")
                vhA = constp.tile(
                    [128, nb, 65], bf16, tag=f"vo{2 * p}", name=f"vo_{2 * p}"
                )
                vhB = constp.tile(
                    [128, nb, 65], bf16, tag=f"vo{2 * p + 1}", name=f"vo_{2 * p + 1}"
                )
                if p == 0:
                    nc.sync.dma_start(out=kT[:, 0:512], in_=kT_p[p, :, 0:512])
                    nc.sync.dma_start(
                        out=q2[:, :, 0:512], in_=q2_p[p, :, :, 0:512]
                    )
                    nc.sync.dma_start(out=vhA[:], in_=vo_p[2 * p])
                    nc.sync.dma_start(out=vhB[:], in_=vo_p[2 * p + 1])
                    nc.sync.dma_start(out=tri[:], in_=tri_p[:])
                    nc.sync.dma_start(out=kT[:, 512:1024], in_=kT_p[p, :, 512:1024])
                    nc.sync.dma_start(
                        out=q2[:, :, 512:1024], in_=q2_p[p, :, :, 512:1024]
                    )
                    nc.sync.dma_start(out=kT[:, 1024:n], in_=kT_p[p, :, 1024:n])
                    nc.sync.dma_start(
                        out=q2[:, :, 1024:n], in_=q2_p[p, :, :, 1024:n]
                    )
                else:
                    nc.sync.dma_start(out=kT[:, 0:nh], in_=kT_p[p, :, 0:nh])
                    nc.sync.dma_start(out=q2[:, :, 0:nh], in_=q2_p[p, :, :, 0:nh])
                    nc.sync.dma_start(out=vhA[:], in_=vo_p[2 * p])
                    nc.sync.dma_start(out=vhB[:], in_=vo_p[2 * p + 1])
                    nc.sync.dma_start(out=kT[:, nh:n], in_=kT_p[p, :, nh:n])
                    nc.sync.dma_start(out=q2[:, :, nh:n], in_=q2_p[p, :, :, nh:n])
                kTs.append(kT)
                q2s.append(q2)
                vos.append((vhA, vhB))

            ptds = []
            for r in range(4):
                ptd = ptdp.tile([128, 2, 512], bf16, tag=f"ptd{r}", name=f"ptd_{r}")
                nc.gpsimd.memset(ptd[:], 0.0)
                ptds.append(ptd)

            jj = 0
            mask_src = nc.gpsimd if mask_engine == "gpsimd" else nc.vector
            for p in range(npairs):
                kT, q2 = kTs[p], q2s[p]
                vhA, vhB = vos[p]
                for t in range(nqt):
                    nblk = 4 * (t + 1)
                    acc = ps_acc.tile(
                        [65, 2, 512], f32, tag="acc", name=f"acc_{p}_{t}"
                    )
                    for j in range(nblk):
                        r = j - 4 * t
                        lo = 128 * r if r >= 0 else 0
                        sp = ps_sp.tile([128, 2, 512], f32, tag="sp")
                        kslc = kT[:, 128 * j : 128 * (j + 1)]
                        qs = slice(512 * t + lo, 512 * (t + 1))
                        nc.tensor.matmul(
                            sp[:, 0, lo:512],
                            kslc,
                            q2[:, 0, qs],
                            start=True,
                            stop=True,
                        )
                        nc.tensor.matmul(
                            sp[:, 1, lo:512],
                            kslc,
                            q2[:, 1, qs],
                            start=True,
                            stop=True,
                        )
                        pt = (
                            ptds[r]
                            if r >= 0
                            else ptop.tile([128, 2, 512], bf16, tag="pt")
                        )
                        if dve_mode[jj % len(dve_mode)] == "A":
                            nc.scalar.activation(
                                pt[:, :, lo:512],
                                sp[:, :, lo:512],
                                Exp,
                                scale=2.0 * SCALE,
                            )
                        else:
                            nc.vector.tensor_scalar(
                                pt[:, :, lo:512].bitcast(u16),
                                sp[:, :, lo:512],
                                TS_A,
                                TS_B,
                                mult,
                                add,
                            )
                        jj += 1
                        if r >= 0:
                            mask_src.tensor_mul(
                                pt[:, :, lo : lo + 128],
                                pt[:, :, lo : lo + 128],
                                tri[:],
                            )
                        nc.tensor.matmul(
                            acc[:, 0, lo:512],
                            vhA[:, j, :],
                            pt[:, 0, lo:512],
                            start=(j == 0),
                            stop=(j == nblk - 1),
                        )
                        nc.tensor.matmul(
                            acc[:, 1, lo:512],
                            vhB[:, j, :],
                            pt[:, 1, lo:512],
                            start=(j == 0),
                            stop=(j == nblk - 1),
                        )
                    acc_sb = finp.tile([65, 2, 512], f32, tag="acc_sb")
                    if (p * nqt + t) % 2 == 0:
                        nc.vector.tensor_copy(acc_sb[:], acc[:])
                    else:
                        nc.scalar.activation(
                            acc_sb[:],
                            acc[:],
                            mybir.ActivationFunctionType.Copy,
                        )
                    nc.sync.dma_start(out=o_p[p, t], in_=acc_sb[:])

    nc.finalize()
    return nc


def get_program(dve_mode="AD", mask_engine="vector"):
    key = (dve_mode, mask_engine)
    if key not in _BUILT:
        _BUILT[key] = _build(dve_mode, mask_engine)
    return _BUILT[key]


def _tri_np():
    import ml_dtypes

    kv = np.arange(128)[:, None]
    c = np.arange(128)[None, :]
    tri = (c >= kv).astype(ml_dtypes.bfloat16)  # [128, 128]
    return np.ascontiguousarray(np.repeat(tri[:, None, :], 2, axis=1))


def make_in_maps(q, k, v):
    """Host-side prep: split + pack full [B,H,N,D] inputs per core."""
    import ml_dtypes

    bf = ml_dtypes.bfloat16
    qf = np.asarray(q, dtype=np.float32).reshape(B * H, N, D)
    kf = np.asarray(k, dtype=np.float32).reshape(B * H, N, D)
    vf = np.asarray(v, dtype=np.float32).reshape(B * H, N, D)

    ksq = np.sum(kf.astype(np.float64) ** 2, axis=-1)       # [BH, N]
    ek = np.exp(-SCALE * ksq).astype(np.float32)            # [BH, N]
    # vo[h, kv%128, kv//128, 0:64] = v*ek ; [..., 64] = ek
    vo = np.empty((B * H, 128, NB, 65), dtype=bf)
    vek = (vf * ek[:, :, None]).reshape(B * H, NB, 128, 64)
    vo[:, :, :, 0:64] = vek.transpose(0, 2, 1, 3).astype(bf)
    vo[:, :, :, 64] = ek.reshape(B * H, NB, 128).transpose(0, 2, 1).astype(bf)

    qT = qf.transpose(0, 2, 1).astype(bf)                   # [BH, 64, N]
    kT = kf.transpose(0, 2, 1).astype(bf)
    tri = _tri_np()

    maps = []
    for c in range(NCORES):
        h0 = c * HPC
        q2 = np.zeros((NPAIRS, 128, 2, N), dtype=bf)
        kt = np.empty((NPAIRS, 128, N), dtype=bf)
        for p in range(NPAIRS):
            hA, hB = h0 + 2 * p, h0 + 2 * p + 1
            q2[p, 0:64, 0] = qT[hA]
            q2[p, 64:128, 1] = qT[hB]
            kt[p, 0:64] = kT[hA]
            kt[p, 64:128] = kT[hB]
        maps.append(
            {
                "qT2": q2,
                "kT": kt,
                "vo": np.ascontiguousarray(vo[h0 : h0 + HPC]),
                "tri": tri,
            }
        )
    return maps


def postprocess(raws):
    """raws: list of per-core [NPAIRS, NQT, 65, 1024] f32 -> [B,H,N,D]."""
    outs = []
    for raw in raws:
        r = raw.reshape(NPAIRS, NQT, 65, 2, 512)
        num = r[:, :, 0:64]                     # [p, t, d, h, iq]
        den = 1.0 + r[:, :, 64]                 # [p, t, h, iq]
        o = num / den[:, :, None]
        # [p, t, d, h, iq] -> [p, h, t, iq, d]
        outs.append(o.transpose(0, 3, 1, 4, 2).reshape(HPC, N, D))
    return np.concatenate(outs, axis=0).astype(np.float32)


def kernel(q, k, v):
    from concourse.bass_utils import run_bass_kernel_spmd

    nc = get_program()
    maps = make_in_maps(q, k, v)
    res = run_bass_kernel_spmd(nc, maps, list(range(NCORES)))
    out = postprocess([res.results[c]["out"] for c in range(NCORES)])
    return out.reshape(B, H, N, D)


# revision 29
# speedup vs baseline: 1.0014x; 1.0014x over previous
"""Trainium2 Bass kernel for nn_Attend (l2-distance attention with zero-kv).

Reference computation (per b,h):
    k' = [0; k], v' = [0; v]                       (prepend zero kv)
    scores[i,j] = (2 q_i.k'_j - |q_i|^2 - |k'_j|^2) * (D+2)^-0.5
    causal: j <= i+1 in padded index space
    out = softmax(scores) @ v'

Kernel algebra: softmax is invariant to the per-row constant -scale*|q_i|^2,
so with p~[i,j] = exp(2*scale*q_i.k_j) * exp(-scale*|k_j|^2) and the zero
column contributing exp(0)=1 to the denominator only:
    out_i = (sum_j p~ v_j) / (1 + sum_j p~)

v4 design (vs v3 baseline at 184us):
  * All input staging moved to the HOST: q^T/k^T prepacked bf16 with the
    two heads of a pair stacked block-diagonally on the partition dim
    (K=128 keeps the PE moving operand at full rate), vo = [v*ek | ek]
    with ek = exp(-scale*|k|^2) folded in, and the causal triangle mask.
    Removes all on-device transposes/casts/copies (~40us DVE/gpsimd/DMA).
  * Scores stay transposed ([kv, q]); for diagonal kv blocks the matmul,
    exp and PV are restricted to columns >= 128*r (the fully-masked strip
    is never computed), with per-r dedicated pre-zeroed pt tiles.
  * exp is split across TWO engines: the Scalar/ACT engine (table exp)
    and the DVE via a one-instruction bit-trick:
        bf16_bits(exp(x)) ~= round(128*(log2e*x + 127 - c))
    written as tensor_scalar(out=uint16 view, in0=scores, mult, add).
  * The triangle mask multiply runs on the DVE right after the exp.
  * No on-device softmax division/transpose: the [65, q] accumulator
    (64 v-dims + denominator row) is DMA'd out raw; the host does
    num/(1+den) and the final [d,q]->[q,d] transpose.

Sharding: 32 (b,h) pairs -> 4 heads per core, 8 cores, pure data parallel.
"""

import sys

for _p in ("/opt/trn_rl_repo", "/root/.axon_site"):
    if _p not in sys.path:
        sys.path.insert(0, _p)

import numpy as np

B, H, N, D = 2, 16, 2048, 64
NCORES = 8
HPC = (B * H) // NCORES          # heads per core = 4
NPAIRS = HPC // 2
SCALE = float((D + 2) ** -0.5)   # augmented head dim, matches reference
NB = N // 128                    # kv blocks of 128 = 16
NQT = N // 512                   # q tiles of 512 = 4

# DVE bit-trick exp constants: bits = TS_A * x + TS_B, viewed as bf16
_C_CORR = 0.04303
TS_A = float(2.0 * SCALE * 128.0 / np.log(2.0))
TS_B = float((127.0 - _C_CORR) * 128.0)

_BUILT = {}


def _build(dve_mode="AD", mask_engine="vector", hpc=HPC, n=N):
    """Build + finalize the SPMD Bass program (one core's view).

    dve_mode: per-j engine pattern for the exp ("A" = ACT table exp,
    "D" = DVE bit-trick), cycled over the global j counter.
    """
    import concourse.mybir as mybir
    import concourse.tile as tile
    from concourse import bacc

    f32 = mybir.dt.float32
    bf16 = mybir.dt.bfloat16
    u16 = mybir.dt.uint16
    Exp = mybir.ActivationFunctionType.Exp
    mult = mybir.AluOpType.mult
    add = mybir.AluOpType.add

    npairs = hpc // 2
    nb = n // 128
    nqt = n // 512
    nh = n // 2  # DMA half size

    nc = bacc.Bacc("TRN2", target_bir_lowering=False, debug=False)
    q2_p = nc.declare_dram_parameter("qT2", [npairs, 128, 2, n], bf16, isOutput=False)
    kT_p = nc.declare_dram_parameter("kT", [npairs, 128, n], bf16, isOutput=False)
    vo_p = nc.declare_dram_parameter("vo", [hpc, 128, nb, 65], bf16, isOutput=False)
    tri_p = nc.declare_dram_parameter("tri", [128, 2, 128], bf16, isOutput=False)
    o_p = nc.declare_dram_parameter("out", [npairs, nqt, 65, 1024], f32, isOutput=True)

    with tile.TileContext(nc) as tc:
        with (
            tc.tile_pool(name="const", bufs=1) as constp,
            tc.tile_pool(name="ptd", bufs=1) as ptdp,
            tc.tile_pool(name="pto", bufs=6) as ptop,
            tc.tile_pool(name="fin", bufs=2) as finp,
            tc.tile_pool(name="ps_sp", bufs=3, space="PSUM") as ps_sp,
            tc.tile_pool(name="ps_acc", bufs=1, space="PSUM") as ps_acc,
        ):
            dwarm = constp.tile([128, 2], f32, tag="dwarm")
            tri = constp.tile([128, 2, 128], bf16, tag="tri")
            kTs, q2s, vos = [], [], []
            # staged DMAs so the first matmuls start early; pair 0 first,
            # first wave spread across queues so configs run in parallel
            for p in range(npairs):
                kT = constp.tile([128, n], bf16, tag=f"kT{p}", name=f"kT_{p}")
                q2 = constp.tile([128, 2, n], bf16, tag=f"q2{p}", name=f"q2_{p}")
                vhA = constp.tile(
                    [128, nb, 65], bf16, tag=f"vo{2 * p}", name=f"vo_{2 * p}"
                )
                vhB = constp.tile(
                    [128, nb, 65], bf16, tag=f"vo{2 * p + 1}", name=f"vo_{2 * p + 1}"
                )
                if p == 0:
                    nc.sync.dma_start(out=kT[:, 0:512], in_=kT_p[p, :, 0:512])
                    nc.scalar.dma_start(
                        out=q2[:, :, 0:512], in_=q2_p[p, :, :, 0:512]
                    )
                    nc.gpsimd.dma_start(out=vhA[:], in_=vo_p[2 * p])
                    nc.gpsimd.dma_start(out=vhB[:], in_=vo_p[2 * p + 1])
                    nc.sync.dma_start(out=tri[:], in_=tri_p[:])
                    # warm the ACT Exp table during the input-DMA wait
                    nc.vector.memset(dwarm[:, 0:1], 0.0)
                    nc.scalar.activation(dwarm[:, 1:2], dwarm[:, 0:1], Exp)
                    nc.sync.dma_start(out=kT[:, 512:1024], in_=kT_p[p, :, 512:1024])
                    nc.sync.dma_start(
                        out=q2[:, :, 512:1024], in_=q2_p[p, :, :, 512:1024]
                    )
                    nc.sync.dma_start(out=kT[:, 1024:n], in_=kT_p[p, :, 1024:n])
                    nc.sync.dma_start(
                        out=q2[:, :, 1024:n], in_=q2_p[p, :, :, 1024:n]
                    )
                else:
                    nc.sync.dma_start(out=kT[:, 0:nh], in_=kT_p[p, :, 0:nh])
                    nc.sync.dma_start(out=q2[:, :, 0:nh], in_=q2_p[p, :, :, 0:nh])
                    nc.sync.dma_start(out=vhA[:], in_=vo_p[2 * p])
                    nc.sync.dma_start(out=vhB[:], in_=vo_p[2 * p + 1])
                    nc.sync.dma_start(out=kT[:, nh:n], in_=kT_p[p, :, nh:n])
                    nc.sync.dma_start(out=q2[:, :, nh:n], in_=q2_p[p, :, :, nh:n])
                kTs.append(kT)
                q2s.append(q2)
                vos.append((vhA, vhB))

            ptds = []
            for r in range(4):
                ptd = ptdp.tile([128, 2, 512], bf16, tag=f"ptd{r}", name=f"ptd_{r}")
                nc.gpsimd.memset(ptd[:], 0.0)
                ptds.append(ptd)

            jj = 0
            mask_src = nc.gpsimd if mask_engine == "gpsimd" else nc.vector
            for p in range(npairs):
                kT, q2 = kTs[p], q2s[p]
                vhA, vhB = vos[p]
                for t in range(nqt):
                    nblk = 4 * (t + 1)
                    acc = ps_acc.tile(
                        [65, 2, 512], f32, tag="acc", name=f"acc_{p}_{t}"
                    )
                    for j in range(nblk):
                        r = j - 4 * t
                        lo = 128 * r if r >= 0 else 0
                        sp = ps_sp.tile([128, 2, 512], f32, tag="sp")
                        kslc = kT[:, 128 * j : 128 * (j + 1)]
                        qs = slice(512 * t + lo, 512 * (t + 1))
                        nc.tensor.matmul(
                            sp[:, 0, lo:512],
                            kslc,
                            q2[:, 0, qs],
                            start=True,
                            stop=True,
                        )
                        nc.tensor.matmul(
                            sp[:, 1, lo:512],
                            kslc,
                            q2[:, 1, qs],
                            start=True,
                            stop=True,
                        )
                        pt = (
                            ptds[r]
                            if r >= 0
                            else ptop.tile([128, 2, 512], bf16, tag="pt")
                        )
                        if dve_mode[jj % len(dve_mode)] == "A":
                            nc.scalar.activation(
                                pt[:, :, lo:512],
                                sp[:, :, lo:512],
                                Exp,
                                scale=2.0 * SCALE,
                            )
                        else:
                            nc.vector.tensor_scalar(
                                pt[:, :, lo:512].bitcast(u16),
                                sp[:, :, lo:512],
                                TS_A,
                                TS_B,
                                mult,
                                add,
                            )
                        jj += 1
                        if r >= 0:
                            mask_src.tensor_mul(
                                pt[:, :, lo : lo + 128],
                                pt[:, :, lo : lo + 128],
                                tri[:],
                            )
                        nc.tensor.matmul(
                            acc[:, 0, lo:512],
                            vhA[:, j, :],
                            pt[:, 0, lo:512],
                            start=(j == 0),
                            stop=(j == nblk - 1),
                        )
                        nc.tensor.matmul(
                            acc[:, 1, lo:512],
                            vhB[:, j, :],
                            pt[:, 1, lo:512],
                            start=(j == 0),
                            stop=(j == nblk - 1),
                        )
                    # split the finalize copy across DVE+ACT so the single
                    # acc psum buffer frees in ~720ns instead of ~1220ns
                    acc_sb = finp.tile([65, 2, 512], f32, tag="acc_sb")
                    nc.vector.tensor_copy(acc_sb[:, :, 0:256], acc[:, :, 0:256])
                    nc.scalar.activation(
                        acc_sb[:, :, 256:512],
                        acc[:, :, 256:512],
                        mybir.ActivationFunctionType.Copy,
                    )
                    nc.sync.dma_start(out=o_p[p, t], in_=acc_sb[:])

    nc.finalize()
    return nc


def get_program(dve_mode="AD", mask_engine="vector"):
    key = (dve_mode, mask_engine)
    if key not in _BUILT:
        _BUILT[key] = _build(dve_mode, mask_engine)
    return _BUILT[key]


def _tri_np():
    import ml_dtypes

    kv = np.arange(128)[:, None]
    c = np.arange(128)[None, :]
    tri = (c >= kv).astype(ml_dtypes.bfloat16)  # [128, 128]
    return np.ascontiguousarray(np.repeat(tri[:, None, :], 2, axis=1))


def make_in_maps(q, k, v):
    """Host-side prep: split + pack full [B,H,N,D] inputs per core."""
    import ml_dtypes

    bf = ml_dtypes.bfloat16
    qf = np.asarray(q, dtype=np.float32).reshape(B * H, N, D)
    kf = np.asarray(k, dtype=np.float32).reshape(B * H, N, D)
    vf = np.asarray(v, dtype=np.float32).reshape(B * H, N, D)

    ksq = np.sum(kf.astype(np.float64) ** 2, axis=-1)       # [BH, N]
    ek = np.exp(-SCALE * ksq).astype(np.float32)            # [BH, N]
    # vo[h, kv%128, kv//128, 0:64] = v*ek ; [..., 64] = ek
    vo = np.empty((B * H, 128, NB, 65), dtype=bf)
    vek = (vf * ek[:, :, None]).reshape(B * H, NB, 128, 64)
    vo[:, :, :, 0:64] = vek.transpose(0, 2, 1, 3).astype(bf)
    vo[:, :, :, 64] = ek.reshape(B * H, NB, 128).transpose(0, 2, 1).astype(bf)

    qT = qf.transpose(0, 2, 1).astype(bf)                   # [BH, 64, N]
    kT = kf.transpose(0, 2, 1).astype(bf)
    tri = _tri_np()

    maps = []
    for c in range(NCORES):
        h0 = c * HPC
        q2 = np.zeros((NPAIRS, 128, 2, N), dtype=bf)
        kt = np.empty((NPAIRS, 128, N), dtype=bf)
        for p in range(NPAIRS):
            hA, hB = h0 + 2 * p, h0 + 2 * p + 1
            q2[p, 0:64, 0] = qT[hA]
            q2[p, 64:128, 1] = qT[hB]
            kt[p, 0:64] = kT[hA]
            kt[p, 64:128] = kT[hB]
        maps.append(
            {
                "qT2": q2,
                "kT": kt,
                "vo": np.ascontiguousarray(vo[h0 : h0 + HPC]),
                "tri": tri,
            }
        )
    return maps


def postprocess(raws):
    """raws: list of per-core [NPAIRS, NQT, 65, 1024] f32 -> [B,H,N,D]."""
    outs = []
    for raw in raws:
        r = raw.reshape(NPAIRS, NQT, 65, 2, 512)
        num = r[:, :, 0:64]                     # [p, t, d, h, iq]
        den = 1.0 + r[:, :, 64]                 # [p, t, h, iq]
        o = num / den[:, :, None]
        # [p, t, d, h, iq] -> [p, h, t, iq, d]
        outs.append(o.transpose(0, 3, 1, 4, 2).reshape(HPC, N, D))
    return np.concatenate(outs, axis=0).astype(np.float32)


def kernel(q, k, v):
    from concourse.bass_utils import run_bass_kernel_spmd

    nc = get_program()
    maps = make_in_maps(q, k, v)
    res = run_bass_kernel_spmd(nc, maps, list(range(NCORES)))
    out = postprocess([res.results[c]["out"] for c in range(NCORES)])
    return out.reshape(B, H, N, D)
